# revision 6
# baseline (speedup 1.0000x reference)
"""ConditionEmbedder kernel for 8 Trainium2 NeuronCores.

Math (train=0, unconditioned=0 path):
    drop = isnan(labels);  safe = where(drop, 0, labels)
    s    = softmax(safe[:,d,None]*w1[d] + b1[d], axis=-1)        # per (b, d)
    mlp  = s @ w2[d].T
    out  = sum_d where(drop, emb_w[d], mlp)                      # [B, H]

Algorithm: the per-(b,d) contribution mlp[b,d,:] is a smooth function
f_d: R -> R^128 of the single scalar labels[b,d].  Each component is fit
at runtime by a degree-13 Chebyshev polynomial on x in [-SFIT, SFIT]
(grid error ~4e-4 against an output scale of ~0.36).  The device then
computes, per sample,
    out[b] = sum_{d,p} c[d,p,:] * T_p(u_{b,d})  + c0_sum + emb fallback
as ONE K=112 fp16 matmul per 512-sample stripe: K = 8 dims x 13 Chebyshev
rows + 8 NaN-fallback indicator rows.  The T_p are built with the fp32
double-and-add recurrence (T_2n = 2 T_n^2 - 1, T_{m+n} = 2 T_m T_n -
T_{m-n}) spread across the vector/scalar/pool engines, converted to fp16,
and DMA-scattered into the (d,p)-partition matmul layout.  |x| > SFIT is
clamped to +-1 on device and those few samples (~0.2%) are patched
exactly on the host.  Output is written fp16 (adds < 2.5e-4 rel) and
upcast on the host, halving the dominant HBM write.

Device strategy (pure data parallel over batch, 8 cores): each core owns
B/8 = 16384 samples; weights/coefficients replicated; no collectives.
"""

import sys

import numpy as np

_B, _D, _H = 131072, 8, 128
_NCORES = 8
_BC = _B // _NCORES          # batch rows per core
_P = 13                      # Chebyshev degree
_SFIT = 3.0                  # fit half-range; |x| > _SFIT patched on host
_KV = _D * _P                # 104 polynomial rows
_K = _KV + _D                # + 8 drop-indicator rows = 112


def _np_reference(labels, emb_w, w1, b1, w2, train, unconditioned):
    """Slow exact fallback for the train/unconditioned branches (uses jax to
    reproduce the reference PRNG streams)."""
    import jax
    import jax.numpy as jnp

    DROPOUT_PROB = 0.1
    labels = jnp.asarray(labels)
    if unconditioned:
        drop = jnp.ones(labels.shape, dtype=bool)
    else:
        drop = jnp.isnan(labels)
        if train:
            rkey = jax.random.fold_in(jax.random.key(0), 1)
            drop = drop | (jax.random.uniform(rkey, labels.shape) < DROPOUT_PROB)
    safe = jnp.where(drop, 0.0, labels)
    h1 = safe[:, :, None] * w1[None, :, :] + b1[None, :, :]
    s = jax.nn.softmax(h1, axis=-1)
    mlp = jnp.einsum('bdh,dkh->bdk', s, w2)
    emb = jnp.where(drop[:, :, None], emb_w[None, :, :], mlp)
    if train:
        nkey = jax.random.fold_in(jax.random.key(0), 2)
        emb = emb + jax.random.normal(nkey, emb.shape, dtype=emb.dtype)
    return np.asarray(emb.sum(axis=1))


def _f_exact(x, w1d, b1d, w2d):
    """Exact f_d(x) = softmax(x*w1d + b1d) @ w2d.T, stable, fp64.
    x: [N]; returns [N, H]."""
    lg = x[:, None] * w1d[None, :].astype(np.float64) + b1d[None, :].astype(np.float64)
    m = lg.max(-1, keepdims=True)
    e = np.exp(lg - m)
    s = e / e.sum(-1, keepdims=True)
    return s @ w2d.astype(np.float64).T


def _fit_coeffs(w1, b1, w2):
    """Per-(d,k) Chebyshev coefficients on [-SFIT, SFIT], fp64 [D, P+1, H]."""
    import numpy.polynomial.chebyshev as CH

    G = 2049
    ug = -np.cos(np.linspace(0.0, np.pi, G))
    xg = _SFIT * ug
    Vmat = CH.chebvander(ug, _P)
    coefs = np.zeros((_D, _P + 1, _H))
    for d in range(_D):
        y = _f_exact(xg, w1[d], b1[d], w2[d])
        coefs[d] = np.linalg.lstsq(Vmat, y, rcond=None)[0]
    return coefs


class _Builder:
    """Builds the per-core Bass program (identical on all cores; data differs)."""

    def __init__(self):
        sys.path.insert(0, '/opt/trn_rl_repo')
        import concourse.mybir as mybir
        from concourse import bass, tile
        from concourse.vector_clock import ScopedClock

        self.mybir = mybir
        self.bass = bass
        self.tile = tile
        self.ScopedClock = ScopedClock

    def make_tile_context(self, nc):
        mybir = self.mybir
        tile = self.tile
        ScopedClock = self.ScopedClock

        class PatchedTileContext(tile.TileContext):
            # walrus in this container rejects >1 sync-wait on the tail Drain
            # (setupSyncWait CTRL limit); spread the end-of-kernel waits
            # across single-wait SP nops instead.
            def _drain_and_barrier(self, tick_clock, wait_clock):
                nc_ = self.nc
                probe = nc_.sync.nop(nofuse=True)
                wait_clock.add_sem_waits(
                    probe.ins, ScopedClock({None: tick_clock.global_clock})
                )
                si = probe.ins.sync_info
                waits = list(si.on_wait) if si and si.on_wait else []
                if len(waits) > 1:
                    si.on_wait.clear()
                    si.on_wait.append(waits[0])
                    for w in waits[1:]:
                        n2 = nc_.sync.nop(nofuse=True)
                        s2 = n2.ins.sync_info
                        if s2 is None:
                            n2.ins.sync_info = mybir.SyncInfo(on_wait=[w], on_update=[])
                        else:
                            s2.on_wait.append(w)
                nc_.sync.drain()
                nc_.all_engine_barrier()
                assert self.sems is not None
                popped = nc_._tile_sem_poison_stack.pop()
                assert popped is self._sem_poison
                nc_.clear_and_free_semaphores(list(self.sems.allocated().values()))
                nc_.all_engine_barrier()

        return PatchedTileContext(nc)

    def build(self, inv_scale):
        mybir = self.mybir
        bass = self.bass
        dt = mybir.dt
        ALU = mybir.AluOpType
        F32, F16, U8 = dt.float32, dt.float16, dt.uint8
        Copy = mybir.ActivationFunctionType.Copy
        Ident = mybir.ActivationFunctionType.Identity

        nc = bass.Bass(trn_type="TRN2", enable_partition_id=False)

        # ---- DRAM parameters ----
        # per-core labels, transposed-dense: row (16d + c) holds
        # labels[c*1024:(c+1)*1024, d] of this core's batch slice.
        p_lab = nc.declare_dram_parameter("lab_td", [128, 1024], F32, isOutput=False)
        # row 8(p-1)+d = cheb coeff c[d,p,:]; rows 104..111 = emb fallback
        p_lhsT = nc.declare_dram_parameter("lhsT", [_K, _H], F16, isOutput=False)
        p_c0 = nc.declare_dram_parameter("c0col", [128, 1], F32, isOutput=False)
        # transposed fp16 output; col = per-core batch index
        p_out = nc.declare_dram_parameter("outT", [128, _BC], F16, isOutput=True)

        from contextlib import ExitStack

        with self.make_tile_context(nc) as tc, ExitStack() as ctx:
            consts = ctx.enter_context(tc.tile_pool(name="consts", bufs=1))
            prep = ctx.enter_context(tc.tile_pool(name="prep", bufs=1))
            pop = ctx.enter_context(tc.tile_pool(name="po", bufs=2, space="PSUM"))
            wpp = ctx.enter_context(tc.tile_pool(name="wp", bufs=1, space="PSUM"))
            obp = ctx.enter_context(tc.tile_pool(name="ob", bufs=3))

            t_lhsT = consts.tile([_K, _H], F16)
            nc.sync.dma_start(t_lhsT[:], p_lhsT[:])
            t_c0 = consts.tile([128, 1], F32)
            nc.sync.dma_start(t_c0[:], p_c0[:])
            t_lab = consts.tile([128, 1024], F32)
            nc.sync.dma_start(t_lab[:], p_lab[:])

            # matmul rhs: rows 8(p-1)+d = T_p(u_d); rows 104..111 = dropf
            t_R = consts.tile([_K, _BC], F16, name="R")

            vec, act, pool = nc.vector, nc.scalar, nc.gpsimd

            # ---- PE warm-up: ramp the tensor engine to full clock while
            # the Chebyshev chain runs (reads t_lhsT so it can't start
            # before the const load; results are discarded).
            wpo = wpp.tile([128, _H], F32, tag="wp", name="warm")
            for _ in range(20):
                nc.tensor.matmul(
                    wpo[:], t_lhsT[:, :], t_lhsT[:, :],
                    start=True, stop=True, skip_group_check=True,
                )

            # ---- full-width [128, 1024] fp16 Chebyshev chain ----
            eq = prep.tile([128, 1024], U8, name="eq")
            vec.tensor_tensor(eq[:], t_lab[:], t_lab[:], ALU.is_equal)
            safe = prep.tile([128, 1024], F32, name="safe")
            pool.memset(safe[:], 0.0)
            vec.copy_predicated(safe[:], eq[:], t_lab[:])
            us = prep.tile([128, 1024], F16, name="us")
            act.activation(us[:], safe[:], Copy, scale=float(inv_scale))

            T = {1: prep.tile([128, 1024], F16, name="T1")}
            vec.tensor_scalar(T[1][:], us[:], -1.0, 1.0, ALU.max, ALU.min)
            dropf = prep.tile([128, 1024], F16, name="dropf")
            pool.tensor_scalar(dropf[:], eq[:], -1.0, 1.0, ALU.mult, ALU.add)

            def emit_scatter(p, src):
                nc.sync.dma_start(t_R[8 * (p - 1):8 * p, :], src[:])

            emit_scatter(1, T[1])
            nc.sync.dma_start(t_R[_KV:_KV + 8, :], dropf[:])
            for n in range(1, 7):
                # T_2n = 2*T_n^2 - 1   (scalar square + gpsimd affine)
                s = prep.tile([128, 1024], F16, name=f"s{n}")
                act.square(s[:], T[n][:])
                T[2 * n] = prep.tile([128, 1024], F16, name=f"T{2 * n}")
                pool.tensor_scalar(T[2 * n][:], s[:], 2.0, -1.0,
                                   ALU.mult, ALU.add)
                emit_scatter(2 * n, T[2 * n])
                # T_{2n+1} = 2*T_n*T_{n+1} - T_1   (both on vector)
                m = prep.tile([128, 1024], F16, name=f"m{n}")
                vec.tensor_tensor(m[:], T[n][:], T[n + 1][:], ALU.mult)
                T[2 * n + 1] = prep.tile([128, 1024], F16, name=f"T{2 * n + 1}")
                vec.scalar_tensor_tensor(T[2 * n + 1][:], m[:], 2.0,
                                         T[1][:], ALU.mult, ALU.subtract)
                emit_scatter(2 * n + 1, T[2 * n + 1])

            # ---- stripe loop: one K=112 matmul per 512 samples ----
            # (GPSIMD cannot read PSUM: copies go on scalar/vector only,
            # 2 stripes per copy to halve instruction count.)
            for q in range(4):
                ob = obp.tile([128, 4096], F16, tag="ob", name=f"ob{q}")
                for g in range(4):
                    po = pop.tile([128, 1024], F32, tag="po", name=f"po{q}{g}")
                    for j in (0, 1):
                        c = 8 * q + 2 * g + j
                        nc.tensor.matmul(
                            po[:, 512 * j:512 * (j + 1)], t_lhsT[:, :],
                            t_R[:, 512 * c:512 * (c + 1)],
                            start=True, stop=True,
                        )
                    dst = ob[:, 1024 * g:1024 * (g + 1)]
                    if g == 2:
                        vec.tensor_scalar_add(dst, po[:], t_c0[:, 0:1])
                    else:
                        act.activation(dst, po[:], Ident, bias=t_c0[:, 0:1])
                osl = slice(4096 * q, 4096 * (q + 1))
                nc.scalar.dma_start(p_out[:, osl], ob[:])

        self._split_multi_waits(nc)
        return nc

    def _split_multi_waits(self, nc, maxw=1):
        """walrus in this container caps sync-waits per instruction at 2;
        move excess waits onto inserted same-engine NoOps."""
        mybir = self.mybir
        for f in nc.m.functions:
            for bb in f.blocks:
                new = []
                changed = False
                for ins in list(bb.instructions):
                    si = ins.sync_info
                    waits = list(si.on_wait) if si and si.on_wait else []
                    if len(waits) > maxw:
                        changed = True
                        extra, keep = waits[:-maxw], waits[-maxw:]
                        for j in range(0, len(extra), maxw):
                            new.append(mybir.InstNoOp(
                                name=f"{ins.name}_sw{j}", engine=ins.engine,
                                sync_info=mybir.SyncInfo(
                                    on_wait=list(extra[j:j + maxw]), on_update=[]),
                                text_hint="split_wait"))
                        si.on_wait.clear()
                        for w in keep:
                            si.on_wait.append(w)
                    new.append(ins)
                if changed:
                    bb.instructions = new


def _prepare_host(labels, emb_w, w1, b1, w2):
    fp16 = np.float16
    coefs = _fit_coeffs(w1, b1, w2)              # [D, P+1, H] fp64
    c16 = coefs[:, 1:, :].astype(fp16)           # [D, P, H]
    c0col = coefs[:, 0, :].sum(0).astype(np.float32).reshape(128, 1)

    # device value at u=0 (dropped entries): c0 + sum_p c16[d,p]*T_p(0)
    Tp0 = np.cos(np.arange(1, _P + 1) * np.pi / 2.0)
    Tp0[np.abs(Tp0) < 0.5] = 0.0
    poly0 = coefs[:, 0, :] + np.einsum('p,dpk->dk', Tp0, c16.astype(np.float64))
    emb_corr = (emb_w.astype(np.float64) - poly0).astype(fp16)   # [D, H]

    lhsT = np.zeros((_K, _H), fp16)
    for d in range(_D):
        for p in range(1, _P + 1):
            lhsT[8 * (p - 1) + d] = c16[d, p - 1]
        lhsT[_KV + d] = emb_corr[d]

    # per-core transposed-dense labels: row 16d + c = labels[c*1024:(c+1)*1024, d]
    lab_td = []
    for cc in range(_NCORES):
        lc = labels[cc * _BC:(cc + 1) * _BC]             # [BC, 8]
        td = lc.reshape(16, 1024, 8).transpose(2, 0, 1).reshape(128, 1024)
        lab_td.append(np.ascontiguousarray(td, dtype=np.float32))

    const_map = {"lhsT": lhsT, "c0col": c0col}
    return lab_td, const_map, coefs, c16


def _patch_host(out, labels, w1, b1, w2, coefs, c16):
    """Exactly fix samples where |x| > SFIT (device clamped u to +-1)."""
    flag = np.isfinite(labels) & (np.abs(labels) > _SFIT)
    if not flag.any():
        return
    bb, dd = np.nonzero(flag)
    xv = labels[bb, dd].astype(np.float64)
    sgn = np.sign(xv)
    fex = np.empty((len(bb), _H))
    for d in np.unique(dd):
        m = dd == d
        fex[m] = _f_exact(xv[m], w1[d], b1[d], w2[d])
    pw = sgn[:, None] ** np.arange(1, _P + 1)[None, :]          # [N, P]
    dev = coefs[dd, 0, :] + np.einsum('np,npk->nk', pw, c16[dd].astype(np.float64))
    np.add.at(out, bb, (fex - dev).astype(np.float32))


def _run_device(labels, emb_w, w1, b1, w2, trace=False):
    lab_td, const_map, coefs, c16 = _prepare_host(labels, emb_w, w1, b1, w2)
    builder = _Builder()
    nc = builder.build(1.0 / _SFIT)

    from concourse.bass_utils import run_bass_kernel_spmd
    in_maps = [{"lab_td": lab_td[c], **const_map} for c in range(_NCORES)]
    res = run_bass_kernel_spmd(
        nc, in_maps, list(range(_NCORES)), trace=trace
    )
    out = np.empty((_B, _H), np.float32)
    for c in range(_NCORES):
        out[c * _BC:(c + 1) * _BC] = res.results[c]["outT"].T.astype(np.float32)
    _patch_host(out, labels, w1, b1, w2, coefs, c16)
    return out, res


def kernel(labels, emb_w, w1, b1, w2, train, unconditioned):
    labels = np.asarray(labels)
    emb_w = np.asarray(emb_w, dtype=np.float32)
    w1 = np.asarray(w1, dtype=np.float32)
    b1 = np.asarray(b1, dtype=np.float32)
    w2 = np.asarray(w2, dtype=np.float32)
    if int(np.asarray(train)) or int(np.asarray(unconditioned)):
        return _np_reference(labels, emb_w, w1, b1, w2,
                             int(np.asarray(train)), int(np.asarray(unconditioned)))
    out, _ = _run_device(labels, emb_w, w1, b1, w2, trace=False)
    return out


# revision 12
# speedup vs baseline: 1.1767x; 1.1767x over previous
"""ConditionEmbedder kernel for 8 Trainium2 NeuronCores.

Math (train=0, unconditioned=0 path):
    drop = isnan(labels);  safe = where(drop, 0, labels)
    s    = softmax(safe[:,d,None]*w1[d] + b1[d], axis=-1)        # per (b, d)
    mlp  = s @ w2[d].T
    out  = sum_d where(drop, emb_w[d], mlp)                      # [B, H]

Algorithm: the per-(b,d) contribution mlp[b,d,:] is a smooth function
f_d: R -> R^128 of the single scalar labels[b,d].  Each component is fit
at runtime by a degree-13 Chebyshev polynomial on x in [-SFIT, SFIT]
(grid error ~4e-4 against an output scale of ~0.36).  The device then
computes, per sample,
    out[b] = sum_{d,p} c[d,p,:] * T_p(u_{b,d})  + c0_sum + emb fallback
as ONE K=112 fp16 matmul per 512-sample stripe: K = 8 dims x 13 Chebyshev
rows + 8 NaN-fallback indicator rows.  The T_p are built with the fp32
double-and-add recurrence (T_2n = 2 T_n^2 - 1, T_{m+n} = 2 T_m T_n -
T_{m-n}) spread across the vector/scalar/pool engines, converted to fp16,
and DMA-scattered into the (d,p)-partition matmul layout.  |x| > SFIT is
clamped to +-1 on device and those few samples (~0.2%) are patched
exactly on the host.  Output is written fp16 (adds < 2.5e-4 rel) and
upcast on the host, halving the dominant HBM write.

Device strategy (pure data parallel over batch, 8 cores): each core owns
B/8 = 16384 samples; weights/coefficients replicated; no collectives.
"""

import sys

import numpy as np

_B, _D, _H = 131072, 8, 128
_NCORES = 8
_BC = _B // _NCORES          # batch rows per core
_P = 13                      # Chebyshev degree
_SFIT = 3.0                  # fit half-range; |x| > _SFIT patched on host
_KV = _D * _P                # 104 polynomial rows
_K = _KV + _D                # + 8 drop-indicator rows = 112


def _np_reference(labels, emb_w, w1, b1, w2, train, unconditioned):
    """Slow exact fallback for the train/unconditioned branches (uses jax to
    reproduce the reference PRNG streams)."""
    import jax
    import jax.numpy as jnp

    DROPOUT_PROB = 0.1
    labels = jnp.asarray(labels)
    if unconditioned:
        drop = jnp.ones(labels.shape, dtype=bool)
    else:
        drop = jnp.isnan(labels)
        if train:
            rkey = jax.random.fold_in(jax.random.key(0), 1)
            drop = drop | (jax.random.uniform(rkey, labels.shape) < DROPOUT_PROB)
    safe = jnp.where(drop, 0.0, labels)
    h1 = safe[:, :, None] * w1[None, :, :] + b1[None, :, :]
    s = jax.nn.softmax(h1, axis=-1)
    mlp = jnp.einsum('bdh,dkh->bdk', s, w2)
    emb = jnp.where(drop[:, :, None], emb_w[None, :, :], mlp)
    if train:
        nkey = jax.random.fold_in(jax.random.key(0), 2)
        emb = emb + jax.random.normal(nkey, emb.shape, dtype=emb.dtype)
    return np.asarray(emb.sum(axis=1))


def _f_exact(x, w1d, b1d, w2d):
    """Exact f_d(x) = softmax(x*w1d + b1d) @ w2d.T, stable, fp64.
    x: [N]; returns [N, H]."""
    lg = x[:, None] * w1d[None, :].astype(np.float64) + b1d[None, :].astype(np.float64)
    m = lg.max(-1, keepdims=True)
    e = np.exp(lg - m)
    s = e / e.sum(-1, keepdims=True)
    return s @ w2d.astype(np.float64).T


def _fit_coeffs(w1, b1, w2):
    """Per-(d,k) Chebyshev coefficients on [-SFIT, SFIT], fp64 [D, P+1, H]."""
    import numpy.polynomial.chebyshev as CH

    G = 2049
    ug = -np.cos(np.linspace(0.0, np.pi, G))
    xg = _SFIT * ug
    Vmat = CH.chebvander(ug, _P)
    coefs = np.zeros((_D, _P + 1, _H))
    for d in range(_D):
        y = _f_exact(xg, w1[d], b1[d], w2[d])
        coefs[d] = np.linalg.lstsq(Vmat, y, rcond=None)[0]
    return coefs


class _Builder:
    """Builds the per-core Bass program (identical on all cores; data differs)."""

    def __init__(self):
        sys.path.insert(0, '/opt/trn_rl_repo')
        import concourse.mybir as mybir
        from concourse import bass, tile
        from concourse.vector_clock import ScopedClock

        self.mybir = mybir
        self.bass = bass
        self.tile = tile
        self.ScopedClock = ScopedClock

    def make_tile_context(self, nc):
        mybir = self.mybir
        tile = self.tile
        ScopedClock = self.ScopedClock

        class PatchedTileContext(tile.TileContext):
            # walrus in this container rejects >1 sync-wait on the tail Drain
            # (setupSyncWait CTRL limit); spread the end-of-kernel waits
            # across single-wait SP nops instead.
            def _drain_and_barrier(self, tick_clock, wait_clock):
                nc_ = self.nc
                probe = nc_.sync.nop(nofuse=True)
                wait_clock.add_sem_waits(
                    probe.ins, ScopedClock({None: tick_clock.global_clock})
                )
                si = probe.ins.sync_info
                waits = list(si.on_wait) if si and si.on_wait else []
                if len(waits) > 1:
                    si.on_wait.clear()
                    si.on_wait.append(waits[0])
                    for w in waits[1:]:
                        n2 = nc_.sync.nop(nofuse=True)
                        s2 = n2.ins.sync_info
                        if s2 is None:
                            n2.ins.sync_info = mybir.SyncInfo(on_wait=[w], on_update=[])
                        else:
                            s2.on_wait.append(w)
                nc_.sync.drain()
                nc_.all_engine_barrier()
                assert self.sems is not None
                popped = nc_._tile_sem_poison_stack.pop()
                assert popped is self._sem_poison
                nc_.clear_and_free_semaphores(list(self.sems.allocated().values()))
                nc_.all_engine_barrier()

        return PatchedTileContext(nc)

    def build(self, inv_scale):
        mybir = self.mybir
        bass = self.bass
        dt = mybir.dt
        ALU = mybir.AluOpType
        F32, F16, U8 = dt.float32, dt.float16, dt.uint8
        Copy = mybir.ActivationFunctionType.Copy
        Ident = mybir.ActivationFunctionType.Identity

        nc = bass.Bass(trn_type="TRN2", enable_partition_id=False)

        # ---- DRAM parameters ----
        # per-core labels, transposed-dense: row (16d + c) holds
        # labels[c*1024:(c+1)*1024, d] of this core's batch slice.
        p_lab = nc.declare_dram_parameter("lab_td", [128, 1024], F32, isOutput=False)
        # row 8(p-1)+d = cheb coeff c[d,p,:]; rows 104..111 = emb fallback
        p_lhsT = nc.declare_dram_parameter("lhsT", [_K, _H], F16, isOutput=False)
        p_c0 = nc.declare_dram_parameter("c0col", [128, 1], F32, isOutput=False)
        # transposed fp16 output; col = per-core batch index
        p_out = nc.declare_dram_parameter("outT", [128, _BC], F16, isOutput=True)

        from contextlib import ExitStack

        with self.make_tile_context(nc) as tc, ExitStack() as ctx:
            consts = ctx.enter_context(tc.tile_pool(name="consts", bufs=1))
            prep = ctx.enter_context(tc.tile_pool(name="prep", bufs=1))
            pop = ctx.enter_context(tc.tile_pool(name="po", bufs=2, space="PSUM"))
            wpp = ctx.enter_context(tc.tile_pool(name="wp", bufs=1, space="PSUM"))
            obp = ctx.enter_context(tc.tile_pool(name="ob", bufs=3))

            t_lhsT = consts.tile([_K, _H], F16)
            nc.sync.dma_start(t_lhsT[:], p_lhsT[:])
            t_c0 = consts.tile([128, 1], F32)
            nc.sync.dma_start(t_c0[:], p_c0[:])
            t_lab = consts.tile([128, 1024], F32)
            nc.sync.dma_start(t_lab[:], p_lab[:])

            # matmul rhs: rows 8(p-1)+d = T_p(u_d); rows 104..111 = dropf
            t_R = consts.tile([_K, _BC], F16, name="R")

            vec, act, pool = nc.vector, nc.scalar, nc.gpsimd

            # ---- full-width [128, 1024] fp16 Chebyshev chain ----
            eq = prep.tile([128, 1024], U8, name="eq")
            vec.tensor_tensor(eq[:], t_lab[:], t_lab[:], ALU.is_equal)
            safe = prep.tile([128, 1024], F32, name="safe")
            pool.memset(safe[:], 0.0)
            vec.copy_predicated(safe[:], eq[:], t_lab[:])
            us = prep.tile([128, 1024], F16, name="us")
            act.activation(us[:], safe[:], Copy, scale=float(inv_scale))

            T = {1: prep.tile([128, 1024], F16, name="T1")}
            vec.tensor_scalar(T[1][:], us[:], -1.0, 1.0, ALU.max, ALU.min)
            dropf = prep.tile([128, 1024], F16, name="dropf")
            pool.tensor_scalar(dropf[:], eq[:], -1.0, 1.0, ALU.mult, ALU.add)

            # scatters round-robin over three DMA queues: a single queue
            # moves only ~130 GB/s for these 2 KB-descriptor transfers.
            # (only SP/Act HWDGE + gpsimd SWDGE can issue DMAs.)
            dmaq = [nc.sync, act, pool]
            qi = [0]

            def emit_scatter(p, src):
                dmaq[qi[0] % 3].dma_start(t_R[8 * (p - 1):8 * p, :], src[:])
                qi[0] += 1

            emit_scatter(1, T[1])
            dmaq[qi[0] % 3].dma_start(t_R[_KV:_KV + 8, :], dropf[:])
            qi[0] += 1
            for n in range(1, 7):
                # T_2n = 2*T_n^2 - 1   (scalar square + gpsimd affine)
                s = prep.tile([128, 1024], F16, name=f"s{n}")
                act.square(s[:], T[n][:])
                T[2 * n] = prep.tile([128, 1024], F16, name=f"T{2 * n}")
                pool.tensor_scalar(T[2 * n][:], s[:], 2.0, -1.0,
                                   ALU.mult, ALU.add)
                emit_scatter(2 * n, T[2 * n])
                # T_{2n+1} = 2*T_n*T_{n+1} - T_1   (both on vector)
                m = prep.tile([128, 1024], F16, name=f"m{n}")
                vec.tensor_tensor(m[:], T[n][:], T[n + 1][:], ALU.mult)
                T[2 * n + 1] = prep.tile([128, 1024], F16, name=f"T{2 * n + 1}")
                vec.scalar_tensor_tensor(T[2 * n + 1][:], m[:], 2.0,
                                         T[1][:], ALU.mult, ALU.subtract)
                emit_scatter(2 * n + 1, T[2 * n + 1])

            # ---- PE warm-up: keyed on T[7] (ready just before the last
            # scatters land) so the tensor engine is at full clock when the
            # stripes start; results discarded.
            wpo = wpp.tile([128, 512], F32, tag="wp", name="warm")
            for _ in range(12):
                nc.tensor.matmul(
                    wpo[:], t_lhsT[:, :], T[7][0:_K, 0:512],
                    start=True, stop=True, skip_group_check=True,
                )

            # ---- stripe loop: one K=112 matmul per 512 samples ----
            # (GPSIMD cannot read PSUM: copies go on scalar/vector only,
            # 2 stripes per copy to halve instruction count.)
            for q in range(4):
                ob = obp.tile([128, 4096], F16, tag="ob", name=f"ob{q}")
                for g in range(4):
                    po = pop.tile([128, 1024], F32, tag="po", name=f"po{q}{g}")
                    for j in (0, 1):
                        c = 8 * q + 2 * g + j
                        nc.tensor.matmul(
                            po[:, 512 * j:512 * (j + 1)], t_lhsT[:, :],
                            t_R[:, 512 * c:512 * (c + 1)],
                            start=True, stop=True,
                        )
                    dst = ob[:, 1024 * g:1024 * (g + 1)]
                    if g % 2 == 0:
                        vec.tensor_scalar_add(dst, po[:], t_c0[:, 0:1])
                    else:
                        act.activation(dst, po[:], Ident, bias=t_c0[:, 0:1])
                osl = slice(4096 * q, 4096 * (q + 1))
                (nc.sync if q % 2 == 0 else act).dma_start(p_out[:, osl], ob[:])

        self._split_multi_waits(nc)
        return nc

    def _split_multi_waits(self, nc, maxw=1):
        """walrus in this container caps sync-waits per instruction at 2;
        move excess waits onto inserted same-engine NoOps."""
        mybir = self.mybir
        for f in nc.m.functions:
            for bb in f.blocks:
                new = []
                changed = False
                for ins in list(bb.instructions):
                    si = ins.sync_info
                    waits = list(si.on_wait) if si and si.on_wait else []
                    if len(waits) > maxw:
                        changed = True
                        extra, keep = waits[:-maxw], waits[-maxw:]
                        for j in range(0, len(extra), maxw):
                            new.append(mybir.InstNoOp(
                                name=f"{ins.name}_sw{j}", engine=ins.engine,
                                sync_info=mybir.SyncInfo(
                                    on_wait=list(extra[j:j + maxw]), on_update=[]),
                                text_hint="split_wait"))
                        si.on_wait.clear()
                        for w in keep:
                            si.on_wait.append(w)
                    new.append(ins)
                if changed:
                    bb.instructions = new


def _prepare_host(labels, emb_w, w1, b1, w2):
    fp16 = np.float16
    coefs = _fit_coeffs(w1, b1, w2)              # [D, P+1, H] fp64
    c16 = coefs[:, 1:, :].astype(fp16)           # [D, P, H]
    c0col = coefs[:, 0, :].sum(0).astype(np.float32).reshape(128, 1)

    # device value at u=0 (dropped entries): c0 + sum_p c16[d,p]*T_p(0)
    Tp0 = np.cos(np.arange(1, _P + 1) * np.pi / 2.0)
    Tp0[np.abs(Tp0) < 0.5] = 0.0
    poly0 = coefs[:, 0, :] + np.einsum('p,dpk->dk', Tp0, c16.astype(np.float64))
    emb_corr = (emb_w.astype(np.float64) - poly0).astype(fp16)   # [D, H]

    lhsT = np.zeros((_K, _H), fp16)
    for d in range(_D):
        for p in range(1, _P + 1):
            lhsT[8 * (p - 1) + d] = c16[d, p - 1]
        lhsT[_KV + d] = emb_corr[d]

    # per-core transposed-dense labels: row 16d + c = labels[c*1024:(c+1)*1024, d]
    lab_td = []
    for cc in range(_NCORES):
        lc = labels[cc * _BC:(cc + 1) * _BC]             # [BC, 8]
        td = lc.reshape(16, 1024, 8).transpose(2, 0, 1).reshape(128, 1024)
        lab_td.append(np.ascontiguousarray(td, dtype=np.float32))

    const_map = {"lhsT": lhsT, "c0col": c0col}
    return lab_td, const_map, coefs, c16


def _patch_host(out, labels, w1, b1, w2, coefs, c16):
    """Exactly fix samples where |x| > SFIT (device clamped u to +-1)."""
    flag = np.isfinite(labels) & (np.abs(labels) > _SFIT)
    if not flag.any():
        return
    bb, dd = np.nonzero(flag)
    xv = labels[bb, dd].astype(np.float64)
    sgn = np.sign(xv)
    fex = np.empty((len(bb), _H))
    for d in np.unique(dd):
        m = dd == d
        fex[m] = _f_exact(xv[m], w1[d], b1[d], w2[d])
    pw = sgn[:, None] ** np.arange(1, _P + 1)[None, :]          # [N, P]
    dev = coefs[dd, 0, :] + np.einsum('np,npk->nk', pw, c16[dd].astype(np.float64))
    np.add.at(out, bb, (fex - dev).astype(np.float32))


def _run_device(labels, emb_w, w1, b1, w2, trace=False):
    lab_td, const_map, coefs, c16 = _prepare_host(labels, emb_w, w1, b1, w2)
    builder = _Builder()
    nc = builder.build(1.0 / _SFIT)

    from concourse.bass_utils import run_bass_kernel_spmd
    in_maps = [{"lab_td": lab_td[c], **const_map} for c in range(_NCORES)]
    res = run_bass_kernel_spmd(
        nc, in_maps, list(range(_NCORES)), trace=trace
    )
    out = np.empty((_B, _H), np.float32)
    for c in range(_NCORES):
        out[c * _BC:(c + 1) * _BC] = res.results[c]["outT"].T.astype(np.float32)
    _patch_host(out, labels, w1, b1, w2, coefs, c16)
    return out, res


def kernel(labels, emb_w, w1, b1, w2, train, unconditioned):
    labels = np.asarray(labels)
    emb_w = np.asarray(emb_w, dtype=np.float32)
    w1 = np.asarray(w1, dtype=np.float32)
    b1 = np.asarray(b1, dtype=np.float32)
    w2 = np.asarray(w2, dtype=np.float32)
    if int(np.asarray(train)) or int(np.asarray(unconditioned)):
        return _np_reference(labels, emb_w, w1, b1, w2,
                             int(np.asarray(train)), int(np.asarray(unconditioned)))
    out, _ = _run_device(labels, emb_w, w1, b1, w2, trace=False)
    return out
